# revision 4
# baseline (speedup 1.0000x reference)
"""BCRNN layer (bidirectional convolutional RNN) on 8 Trainium2 NeuronCores.

Problem: nb=1, nc=2, nt=12, nx=160, ny=160, hid=64, K=3, reflect padding,
complex conv decomposed into real convs, modReLU activation, forward +
backward temporal scans, output = sum of the two scans stacked (r, i).

Sharding: spatial rows (nx) split 8 ways (20 rows/core); temporal scans run
locally.  Halo exchange runs every SECOND step (width-2 halos): step A
computes the 20 owned rows, an AllGather + per-core indirect gather then
extends the state to +-2 rows, step B computes 22 rows (one redundant row
each side), and a small local DRAM bounce re-applies the global-edge
reflect rows so no second collective is needed.

v2 schedule: conv groups are ordered EARLY (depend only on the previous
step's interior rows) / MID / LATE (need the AllGather halo or the reflect
refresh), so the PE always has halo-independent matmuls to chew while a
collective is in flight.  modReLU runs as three fused span-chains per step
(rows 5..16 / 2..4 / 17..21 for A-steps) with engines split: DVE does the
psum drains (z = conv + pre) and the final h = max(q,0)*z, Act does the
squares and 1/|z|, Pool (gpsimd) does q = 1 + b/|z|, edge-reflect fixups
and the f+b output combine.  The |z|^2 cross-half pairing stays a PE
matmul (pmat) because two-input DVE/Pool ops require equal base
partitions.  The f-dir square is fp16 at (z/64)^2 (fp16 range), the b-dir
square runs on DVE in bf16 (no scaling needed, f32 psum absorbs the
range).
"""
import os

import numpy as np

import concourse.bass as bass
import concourse.bacc as bacc
import concourse.tile as tile
import concourse.mybir as mybir
from concourse.bass_utils import run_bass_kernel_spmd

P = 128
NC_CORES = 8
NT = 12
NX, NY = 160, 160
HID = 64
ROWS = NX // NC_CORES      # 20 owned rows per core
TR = ROWS + 4              # 24 tile rows (2-row halo each side)
PRE_R = ROWS + 2           # 22 pre rows (one redundant row each side)
YP = NY + 2                # 162 padded cols
FS = TR * YP + 2           # flat h tile size (+1 guard col each side)
ZW = PRE_R * YP            # 3564: z/pre col space, col of tile row r = (r-1)*YP
NSLOT = 12                 # AG slots: 6 per direction
RSLOT = 8                  # local refresh slots: 4 per direction

f32 = mybir.dt.float32
f16 = mybir.dt.float16
bf16 = mybir.dt.bfloat16
i32 = mybir.dt.int32
AF = mybir.ActivationFunctionType
ALU = mybir.AluOpType

# conv groups (tile row q0, nrows) per step kind, in emission order.
# EARLY need only the previous step's early-span rows; MID need its late
# chains; LATE additionally need the AllGather halo (B) or refresh (A).
A_EARLY = [(5, 3), (8, 3), (11, 3), (14, 3)]
A_MID = [(17, 3)]
A_LATE = [(2, 3), (20, 2)]
B_EARLY = [(7, 3), (10, 3), (13, 3)]
B_MID = [(4, 3), (16, 3)]
B_LATE = [(1, 3), (19, 3), (22, 1)]
# modrelu chain spans (row r0, row r1) per step kind
A_SPAN_E, A_SPAN_LL, A_SPAN_LH = (5, 17), (2, 5), (17, 22)
B_SPAN_E, B_SPAN_LL, B_SPAN_LH = (4, 19), (1, 4), (19, 23)

PRE_GROUPS = [(0, 3), (3, 3), (6, 3), (9, 3), (12, 3), (15, 3), (18, 3), (21, 1)]
AG_ROWS = [2, 3, 4, 19, 20, 21]    # contributed h_A tile rows per direction
RF_ROWS = [1, 3, 20, 22]           # contributed h_B tile rows per direction
GN = 3 * YP

_CACHED = {}
TRACE = False
LAST = {}


def _row(q):
    """flat offset of (tile row q, padded col 0) in an FS-sized tile"""
    return 1 + q * YP


def _build():
    if "nc" in _CACHED:
        return _CACHED["nc"]
    nc = bacc.Bacc(None, target_bir_lowering=False, debug=False,
                   num_devices=NC_CORES)

    # ---- I/O ----
    iter_il = nc.dram_tensor("iter_il", [NT, P, TR * YP], f16, kind="ExternalInput")
    im2col = nc.dram_tensor("im2col", [NT, 36, ZW], f16, kind="ExternalInput")
    wh2h = nc.dram_tensor("wh2h", [9, P, P], f16, kind="ExternalInput")
    wih = nc.dram_tensor("wih", [9, P, P], f16, kind="ExternalInput")
    wi2h = nc.dram_tensor("wi2h", [36, P], f16, kind="ExternalInput")
    pmat = nc.dram_tensor("pmat", [P, P], f16, kind="ExternalInput")
    b_pre = nc.dram_tensor("b_pre", [P, 1], f32, kind="ExternalInput")
    b_mod = nc.dram_tensor("b_mod", [P, 1], f32, kind="ExternalInput")
    hidx = nc.dram_tensor("hidx", [P, 8], i32, kind="ExternalInput")
    ridx = nc.dram_tensor("ridx", [P, 4], i32, kind="ExternalInput")
    out = nc.dram_tensor("out", [NT, P, ROWS, NY], f32, kind="ExternalOutput")

    # ---- internal DRAM ----
    predr = nc.dram_tensor("predr", [NT, P, ZW], f16)
    sav = nc.dram_tensor("sav", [NT, P, ROWS, NY], f16)

    with tile.TileContext(nc) as tc:
        with (
            tc.tile_pool(name="wp", bufs=1) as wp,
            tc.tile_pool(name="dram", bufs=2, space="DRAM") as dram,
            tc.tile_pool(name="cps", bufs=2, space="PSUM") as cps,
            tc.tile_pool(name="pps", bufs=1, space="PSUM") as pps,
            tc.tile_pool(name="prps", bufs=2, space="PSUM") as prps,
        ):
            # weights / constants
            wh = [wp.tile([P, P], f16, tag=f"wh{k}", name=f"wh{k}") for k in range(9)]
            wi = [wp.tile([P, P], f16, tag=f"wi{k}", name=f"wi{k}") for k in range(9)]
            for k in range(9):
                nc.sync.dma_start(out=wh[k][:], in_=wh2h[k])
                nc.sync.dma_start(out=wi[k][:], in_=wih[k])
            w36 = wp.tile([36, P], f16, tag="w36")
            nc.sync.dma_start(out=w36[:], in_=wi2h[:])
            pm = wp.tile([P, P], f16, tag="pm")
            nc.sync.dma_start(out=pm[:], in_=pmat[:])
            bp = wp.tile([P, 1], f32, tag="bp")
            nc.sync.dma_start(out=bp[:], in_=b_pre[:])
            bm = wp.tile([P, 1], f32, tag="bm")
            nc.sync.dma_start(out=bm[:], in_=b_mod[:])
            hix = wp.tile([P, 8], i32, tag="hix")
            nc.sync.dma_start(out=hix[:], in_=hidx[:])
            rix = wp.tile([P, 4], i32, tag="rix")
            nc.sync.dma_start(out=rix[:], in_=ridx[:])
            epst = wp.tile([P, 1], f32, tag="epst")
            nc.vector.memset(epst[:], 1e-6)

            # ---------------- phase 1: pre[t] = ih(iter) + i2h(input) + bias
            with (
                tc.tile_pool(name="itp", bufs=2) as itp,
                tc.tile_pool(name="icp", bufs=2) as icp,
                tc.tile_pool(name="prs", bufs=2) as prs,
            ):
                for t in range(NT):
                    it = itp.tile([P, FS], f16, tag="it")
                    nc.sync.dma_start(out=it[:, 1:1 + TR * YP], in_=iter_il[t])
                    ic = icp.tile([36, ZW], f16, tag="ic")
                    nc.sync.dma_start(out=ic[:], in_=im2col[t])
                    stage = prs.tile([P, ZW], f16, tag="stage")
                    for p0, nr in PRE_GROUPS:
                        n = nr * YP
                        ps = prps.tile([P, GN], f32, tag="cvp")
                        for tap in range(9):
                            dxi, dyi = divmod(tap, 3)
                            base = (p0 + dxi) * YP + dyi
                            nc.tensor.matmul(
                                out=ps[:, :n],
                                lhsT=wi[tap][:],
                                rhs=it[:, base:base + n],
                                start=(tap == 0), stop=False,
                            )
                        nc.tensor.matmul(
                            out=ps[:, :n],
                            lhsT=w36[:],
                            rhs=ic[:, p0 * YP:p0 * YP + n],
                            start=False, stop=True,
                        )
                        nc.scalar.activation(stage[:, p0 * YP:p0 * YP + n],
                                             ps[:, :n],
                                             AF.Identity, bias=bp[:, 0:1], scale=1.0)
                    nc.sync.dma_start(out=predr[t], in_=stage[:])

            # ---------------- phase 2: bidirectional scan, exchange every 2
            with (
                tc.tile_pool(name="hfp", bufs=2) as hfp,
                tc.tile_pool(name="hbp", bufs=2) as hbp,
                tc.tile_pool(name="pin", bufs=2) as pinp,
                tc.tile_pool(name="zp", bufs=2) as zp,
                tc.tile_pool(name="z2p", bufs=3) as z2p,
                tc.tile_pool(name="rsp", bufs=1) as rsp,
                tc.tile_pool(name="qp", bufs=1) as qp,
                tc.tile_pool(name="svp", bufs=2) as svp,
                tc.tile_pool(name="obp", bufs=2) as obp,
            ):
                def new_states(s):
                    nexts, pins = {}, {}
                    for dire, hpool in (("f", hfp), ("b", hbp)):
                        t_d = s if dire == "f" else NT - 1 - s
                        pin = pinp.tile([P, ZW], f16, tag=f"pin{dire}",
                                        name=f"pin{dire}")
                        nc.sync.dma_start(out=pin[:], in_=predr[t_d])
                        pins[dire] = pin
                        nexts[dire] = hpool.tile([P, FS], f16, tag=f"h{dire}",
                                                 name=f"hn{dire}")
                    z = zp.tile([P, 2, ZW], f16, tag="z", name="z")
                    rs = rsp.tile([P, 2, ZW], f16, tag="rs", name="rs")
                    q = qp.tile([P, 2, ZW], f16, tag="q", name="q")
                    return nexts, pins, z, rs, q

                def conv_drain(s, q0, nr, prevs, pins, z):
                    """f/b tap-interleaved convs into one 1-bank psum group,
                    then DVE drains z = conv + pre."""
                    n = nr * YP
                    zo = (q0 - 1) * YP
                    pss = {}
                    if s > 0:
                        for dire in ("f", "b"):
                            pss[dire] = cps.tile([P, GN], f32, tag=f"cv{dire}",
                                                 name="cv")
                        for tap in range(9):
                            dxi, dyi = divmod(tap, 3)
                            base = (q0 - 1 + dxi) * YP + dyi
                            for dire in ("f", "b"):
                                nc.tensor.matmul(
                                    out=pss[dire][:, :n],
                                    lhsT=wh[tap][:],
                                    rhs=prevs[dire][:, base:base + n],
                                    start=(tap == 0), stop=(tap == 8),
                                )
                    for di, dire in ((0, "f"), (1, "b")):
                        if s > 0:
                            nc.vector.tensor_tensor(
                                out=z[:, di, zo:zo + n], in0=pss[dire][:, :n],
                                in1=pins[dire][:, zo:zo + n], op=ALU.add)
                        else:
                            nc.vector.tensor_copy(
                                out=z[:, di, zo:zo + n],
                                in_=pins[dire][:, zo:zo + n])

                def sq_pm_rs(q0, nr, z, rs):
                    """squares (f: Act (z/64)^2 fp16, b: DVE bf16), |z|^2
                    pairing pm matmul, rs = 1/|z| — emitted a batch behind
                    the convs so the PE never waits on the drain chain."""
                    n = nr * YP
                    zo = (q0 - 1) * YP
                    z2f = z2p.tile([P, GN], f16, tag="z2f", name="z2f")
                    z2b = z2p.tile([P, GN], bf16, tag="z2b", name="z2b")
                    nc.scalar.activation(z2f[:, :n], z[:, 0, zo:zo + n],
                                         AF.Square, scale=0.015625)
                    nc.vector.tensor_tensor(out=z2b[:, :n], in0=z[:, 1, zo:zo + n],
                                            in1=z[:, 1, zo:zo + n], op=ALU.mult)
                    for di, z2t, scale in ((0, z2f, 4096.0), (1, z2b, 1.0)):
                        mp = pps.tile([P, GN], f32, tag=f"mp{di}", name="mp")
                        nc.tensor.matmul(out=mp[:, :n], lhsT=pm[:],
                                         rhs=z2t[:, :n], start=True, stop=True)
                        nc.scalar.activation(rs[:, di, zo:zo + n], mp[:, :n],
                                             AF.Abs_reciprocal_sqrt,
                                             bias=epst[:, 0:1], scale=scale)

                def chain_span(s, r0, r1, z, rs, q, nexts):
                    """q = 1 + b/|z| (Pool), h = max(q,0)*z (DVE), fixups."""
                    n = (r1 - r0) * YP
                    zo = (r0 - 1) * YP
                    for di, dire in ((0, "f"), (1, "b")):
                        nc.gpsimd.tensor_scalar(
                            out=q[:, di, zo:zo + n], in0=rs[:, di, zo:zo + n],
                            scalar1=bm[:, 0:1], scalar2=1.0,
                            op0=ALU.mult, op1=ALU.add)
                        hb_ = _row(r0)
                        nc.vector.scalar_tensor_tensor(
                            out=nexts[dire][:, hb_:hb_ + n],
                            in0=q[:, di, zo:zo + n], scalar=0.0,
                            in1=z[:, di, zo:zo + n],
                            op0=ALU.max, op1=ALU.mult)
                        if s < NT - 1:
                            vr = nexts[dire][:, hb_:hb_ + n].rearrange(
                                "p (r y) -> p r y", y=YP)
                            eng = nc.vector if dire == "f" else nc.gpsimd
                            eng.tensor_copy(out=vr[:, :, 0:1], in_=vr[:, :, 2:3])
                            eng.tensor_copy(out=vr[:, :, YP - 1:YP],
                                            in_=vr[:, :, YP - 3:YP - 2])

                def save_combine(s, nexts):
                    def _ivw(h_n, r0, nr_):
                        return h_n[:, _row(r0):_row(r0 + nr_)].rearrange(
                            "p (r y) -> p r y", y=YP)[:, :, 1:1 + NY]
                    for dire, t_o in (("f", s), ("b", NT - 1 - s)):
                        h_n = nexts[dire]
                        if s <= 5:
                            nc.sync.dma_start(out=sav[t_o], in_=_ivw(h_n, 2, ROWS))
                        else:
                            for hf_ in range(2):
                                r0 = 2 + hf_ * (ROWS // 2)
                                ld = svp.tile([P, ROWS // 2, NY], f16, tag="ld",
                                              name=f"ld{dire}")
                                nc.sync.dma_start(
                                    out=ld[:],
                                    in_=sav[t_o, :, hf_ * (ROWS // 2):(hf_ + 1) * (ROWS // 2)])
                                ob = obp.tile([P, ROWS // 2, NY], f32, tag="ob",
                                              name=f"ob{dire}")
                                nc.gpsimd.tensor_tensor(
                                    out=ob[:], in0=_ivw(h_n, r0, ROWS // 2),
                                    in1=ld[:], op=ALU.add)
                                nc.sync.dma_start(
                                    out=out[t_o, :, hf_ * (ROWS // 2):(hf_ + 1) * (ROWS // 2)],
                                    in_=ob[:])

                npair = NT // 2
                prevs = {"f": None, "b": None}
                for pair in range(npair):
                    sA, sB = 2 * pair, 2 * pair + 1

                    # ---- A-step
                    nexts_A, pins_A, z_A, rs_A, q_A = new_states(sA)
                    for q0, nr in A_EARLY:
                        conv_drain(sA, q0, nr, prevs, pins_A, z_A)
                    for q0, nr in A_MID + A_LATE:
                        conv_drain(sA, q0, nr, prevs, pins_A, z_A)
                    for q0, nr in A_EARLY:
                        sq_pm_rs(q0, nr, z_A, rs_A)
                    chain_span(sA, *A_SPAN_E, z_A, rs_A, q_A, nexts_A)
                    for q0, nr in A_MID + A_LATE:
                        sq_pm_rs(q0, nr, z_A, rs_A)
                    chain_span(sA, *A_SPAN_LL, z_A, rs_A, q_A, nexts_A)
                    chain_span(sA, *A_SPAN_LH, z_A, rs_A, q_A, nexts_A)
                    save_combine(sA, nexts_A)

                    # ---- exchange: extend h_A to +-2 halo rows
                    cc_in = dram.tile([NSLOT * P, YP], f16, tag="cci", name="cci")
                    cc_out = dram.tile([NC_CORES * NSLOT * P, YP], f16,
                                       addr_space="Shared", tag="cco", name="cco")
                    cci_v = cc_in[:].rearrange("(s p) y -> s p y", p=P)
                    for di, dire in ((0, "f"), (1, "b")):
                        h_n = nexts_A[dire]
                        for si, row in enumerate(AG_ROWS):
                            nc.sync.dma_start(
                                out=cci_v[di * len(AG_ROWS) + si],
                                in_=h_n[:, _row(row):_row(row + 1)])
                    nc.gpsimd.collective_compute(
                        "AllGather", ALU.bypass,
                        replica_groups=[list(range(NC_CORES))],
                        ins=[cc_in[:].opt()], outs=[cc_out[:].opt()],
                    )

                    # ---- B-step: early/mid groups are halo-independent
                    nexts_B, pins_B, z_B, rs_B, q_B = new_states(sB)
                    for q0, nr in B_EARLY + B_MID:
                        conv_drain(sB, q0, nr, nexts_A, pins_B, z_B)
                    for q0, nr in B_EARLY + B_MID:
                        sq_pm_rs(q0, nr, z_B, rs_B)
                    chain_span(sB, *B_SPAN_E, z_B, rs_B, q_B, nexts_B)

                    # halo gathers (wait on AG)
                    for k, (dire, row) in enumerate(
                            (("f", 0), ("f", 1), ("f", TR - 2), ("f", TR - 1),
                             ("b", 0), ("b", 1), ("b", TR - 2), ("b", TR - 1))):
                        h_n = nexts_A[dire]
                        nc.gpsimd.indirect_dma_start(
                            out=h_n[:, _row(row):_row(row + 1)],
                            out_offset=None,
                            in_=cc_out[:],
                            in_offset=bass.IndirectOffsetOnAxis(
                                ap=hix[:, k:k + 1], axis=0),
                        )

                    for q0, nr in B_LATE:
                        conv_drain(sB, q0, nr, nexts_A, pins_B, z_B)
                    for q0, nr in B_LATE:
                        sq_pm_rs(q0, nr, z_B, rs_B)
                    chain_span(sB, *B_SPAN_LL, z_B, rs_B, q_B, nexts_B)
                    chain_span(sB, *B_SPAN_LH, z_B, rs_B, q_B, nexts_B)
                    save_combine(sB, nexts_B)

                    # ---- local reflect refresh of rows 1 and 22
                    if pair < npair - 1:
                        rf = dram.tile([RSLOT * P, YP], f16, tag="rf", name="rf")
                        rf_v = rf[:].rearrange("(s p) y -> s p y", p=P)
                        for di, dire in ((0, "f"), (1, "b")):
                            h_n = nexts_B[dire]
                            for si, row in enumerate(RF_ROWS):
                                nc.sync.dma_start(
                                    out=rf_v[di * 4 + si],
                                    in_=h_n[:, _row(row):_row(row + 1)])
                        for k, (dire, row) in enumerate(
                                (("f", 1), ("f", TR - 2),
                                 ("b", 1), ("b", TR - 2))):
                            h_n = nexts_B[dire]
                            nc.gpsimd.indirect_dma_start(
                                out=h_n[:, _row(row):_row(row + 1)],
                                out_offset=None,
                                in_=rf[:],
                                in_offset=bass.IndirectOffsetOnAxis(
                                    ap=rix[:, k:k + 1], axis=0),
                            )

                    prevs = nexts_B

    nc.compile()
    _CACHED["nc"] = nc
    return nc


def _complex_lhsT(wr, wi_):
    """[O, I, 3, 3] complex pair -> per-tap lhsT [9, 2*I, 2*O]."""
    O, I = wr.shape[:2]
    lhsT = np.zeros((9, 2 * I, 2 * O), np.float32)
    for tap in range(9):
        kx, ky = divmod(tap, 3)
        lhsT[tap, :I, :O] = wr[:, :, kx, ky].T
        lhsT[tap, I:, :O] = -wi_[:, :, kx, ky].T
        lhsT[tap, :I, O:] = wi_[:, :, kx, ky].T
        lhsT[tap, I:, O:] = wr[:, :, kx, ky].T
    return lhsT


def kernel(**inputs):
    inp_r = np.asarray(inputs["input_r"], np.float32)
    inp_i = np.asarray(inputs["input_i"], np.float32)
    itr_r = np.asarray(inputs["iter_r"], np.float32)
    itr_i = np.asarray(inputs["iter_i"], np.float32)

    # ---- weights ----
    wh2h = _complex_lhsT(np.asarray(inputs["w_h2h_r"]), np.asarray(inputs["w_h2h_i"]))
    wih = _complex_lhsT(np.asarray(inputs["w_ih_r"]), np.asarray(inputs["w_ih_i"]))
    w4 = _complex_lhsT(np.asarray(inputs["w_i2h_r"]), np.asarray(inputs["w_i2h_i"]))
    wi2h = np.ascontiguousarray(w4.reshape(36, P))
    pmat = np.zeros((P, P), np.float32)
    for k in range(P):
        pmat[k, k % HID] = 1.0
        pmat[k, HID + k % HID] = 1.0
    b_pre = np.concatenate([
        inputs["b_i2h_r"] + inputs["b_ih_r"] + inputs["b_h2h_r"],
        inputs["b_i2h_i"] + inputs["b_ih_i"] + inputs["b_h2h_i"],
    ]).astype(np.float32)[:, None]
    b_mod = np.tile(np.asarray(inputs["mod_b"], np.float32), 2)[:, None]

    # ---- activations, reflect-padded by 2: index x+2 <-> global row x ----
    itg = np.concatenate([itr_r[0], itr_i[0]], axis=0).transpose(1, 0, 2, 3)
    itg = np.pad(itg, ((0, 0), (0, 0), (2, 2), (2, 2)), mode="reflect")
    ing = np.concatenate([inp_r[0], inp_i[0]], axis=0).transpose(1, 0, 2, 3)
    ing = np.pad(ing, ((0, 0), (0, 0), (2, 2), (2, 2)), mode="reflect")

    in_maps = []
    for c in range(NC_CORES):
        a = c * ROWS
        iter_il = np.ascontiguousarray(
            itg[:, :, a:a + TR, 1:1 + YP]).reshape(NT, P, TR * YP)
        im2col = np.empty((NT, 36, PRE_R, YP), np.float32)
        for tap in range(9):
            kx, ky = divmod(tap, 3)
            for c4 in range(4):
                im2col[:, tap * 4 + c4] = ing[:, c4, a + kx:a + kx + PRE_R,
                                              ky:ky + YP]
        hidxa = np.zeros((P, 8), np.int32)
        ridxa = np.zeros((P, 4), np.int32)
        pa = np.arange(P)

        def agfl(di, rank, row):
            return (rank * NSLOT + di * len(AG_ROWS) + AG_ROWS.index(row)) * P + pa

        for di in (0, 1):
            o = di * 4
            if c == 0:
                hidxa[:, o + 0] = agfl(di, 0, 4)       # reflect of global 2
                hidxa[:, o + 1] = agfl(di, 0, 3)       # reflect of global 1
            else:
                hidxa[:, o + 0] = agfl(di, c - 1, 20)  # global a-2
                hidxa[:, o + 1] = agfl(di, c - 1, 21)  # global a-1
            if c == NC_CORES - 1:
                hidxa[:, o + 2] = agfl(di, c, 20)      # reflect of global 158
                hidxa[:, o + 3] = agfl(di, c, 19)      # reflect of global 157
            else:
                hidxa[:, o + 2] = agfl(di, c + 1, 2)   # global a+20
                hidxa[:, o + 3] = agfl(di, c + 1, 3)   # global a+21

        def rfl(di, row):
            return (di * 4 + RF_ROWS.index(row)) * P + pa

        for di in (0, 1):
            o = di * 2
            ridxa[:, o + 0] = rfl(di, 3) if c == 0 else rfl(di, 1)
            ridxa[:, o + 1] = (rfl(di, 20) if c == NC_CORES - 1
                               else rfl(di, 22))
        in_maps.append({
            "iter_il": iter_il.astype(np.float16),
            "im2col": im2col.reshape(NT, 36, ZW).astype(np.float16),
            "wh2h": wh2h.astype(np.float16), "wih": wih.astype(np.float16),
            "wi2h": wi2h.astype(np.float16), "pmat": pmat.astype(np.float16),
            "b_pre": b_pre, "b_mod": b_mod, "hidx": hidxa, "ridx": ridxa,
        })

    nc = _build()
    try:
        res = run_bass_kernel_spmd(nc, in_maps,
                                   core_ids=list(range(NC_CORES)), trace=TRACE)
    except Exception:
        # transient NRT device-state failures recover on retry
        res = run_bass_kernel_spmd(nc, in_maps,
                                   core_ids=list(range(NC_CORES)), trace=TRACE)
    LAST["exec_time_ns"] = res.exec_time_ns
    LAST["results"] = res

    full = np.empty((1, HID, NT, NX, NY, 2), np.float32)
    for c in range(NC_CORES):
        a = c * ROWS
        o = res.results[c]["out"]          # [NT, 128, ROWS, NY]
        full[0, :, :, a:a + ROWS, :, 0] = o[:, :HID].transpose(1, 0, 2, 3)
        full[0, :, :, a:a + ROWS, :, 1] = o[:, HID:].transpose(1, 0, 2, 3)
    return full
